# revision 1
# baseline (speedup 1.0000x reference)
"""Trainium2 Bass kernel for nn_CapsuleSubLayer (capsule routing).

Math (per head h):
  uh[b,d,j] = sum_s W[h,d,j,s] * x[h,b,s,d]            (batched matmul over d)
  3 routing iterations of softmax / weighted-sum / squash / logit update
  out[b,d,n,h] = v[h,b,d]  (broadcast over n)

Sharding: heads are fully independent -> 2 heads per NeuronCore on 8 cores.
Host-side we pre-permute x and W into DMA-friendly per-(h,d) layouts (and
cast to bf16; PSUM accumulation stays fp32):
  xt[h,d,p,c*64+b] = x[h,b,c*128+p,d]
  wt[h,d,p,c*16+n] = W[h,d,n,c*128+p]
so each per-(h,d) matmul chunk is lhsT=[p,b] (stationary), rhs=[p,n] (moving),
accumulated over c=0..7 into PSUM out[b,n].

Routing runs with partition=b (64 lanes) in fp32, in chunks of 32 d's so it
overlaps the next chunk's DMA/matmul: reductions over n are inner-free
reduces; the mean over b uses a ones-matmul on the PE which also leaves the
result replicated across partitions (exactly the layout the next softmax
needs).
"""

import os
import sys

import numpy as np

for _p in ("/opt/trn_rl_repo",):
    if _p not in sys.path:
        sys.path.insert(0, _p)

from contextlib import ExitStack

import ml_dtypes

import concourse.bass as bass
import concourse.tile as tile
from concourse import bacc, mybir
from concourse.bass_utils import run_bass_kernel_spmd

F32 = mybir.dt.float32
BF16 = mybir.dt.bfloat16

H, B, S, D, N = 16, 64, 1024, 64, 16
NCORES = 8
H_LOC = H // NCORES  # 2 heads per core
C = S // 128  # 8 contraction chunks

USE_FP32 = bool(int(os.environ.get("KERNEL_FP32", "0")))
IN_DT = F32 if USE_FP32 else mybir.dt.float16
IN_NP = np.float32 if USE_FP32 else np.float16

_cache = {}


def _build(num_routing: int, repeat: int = 1):
    nc = bacc.Bacc(
        "TRN2", target_bir_lowering=False, debug=False, num_devices=NCORES
    )
    xt = nc.dram_tensor("xt", [H_LOC, D, 128, C * B], IN_DT, kind="ExternalInput").ap()
    wt = nc.dram_tensor("wt", [H_LOC, D, 128, C * N], IN_DT, kind="ExternalInput").ap()
    ones = nc.dram_tensor("ones", [B, B], mybir.dt.float16, kind="ExternalInput").ap()
    vout = nc.dram_tensor("vout", [B, H_LOC * D], F32, kind="ExternalOutput").ap()

    DG = 8  # d's per DMA batch
    RC = 32  # d's per routing chunk (= PSUM bank group)

    with ExitStack() as ctx:
        tc = ctx.enter_context(tile.TileContext(nc))
        xpool = ctx.enter_context(tc.tile_pool(name="xp", bufs=3))
        wpool = ctx.enter_context(tc.tile_pool(name="wp", bufs=3))
        pspool = ctx.enter_context(tc.tile_pool(name="ps", bufs=3, space="PSUM"))
        bppool = ctx.enter_context(tc.tile_pool(name="bp", bufs=2, space="PSUM"))
        uhpool = ctx.enter_context(tc.tile_pool(name="uh", bufs=3))
        rpool = ctx.enter_context(tc.tile_pool(name="rt", bufs=3))
        spool = ctx.enter_context(tc.tile_pool(name="sm", bufs=6))
        singles = ctx.enter_context(tc.tile_pool(name="sg", bufs=1))

        ones_sb = singles.tile([B, B], mybir.dt.float16)
        nc.sync.dma_start(out=ones_sb, in_=ones)

        def routing(uh, vout_slice, RC):
            """3-iteration dynamic routing on a [B, RC, N] uh chunk.
            b_logits live in PSUM (bl_ps), accumulated by ones*N/B matmuls."""
            bl_ps = bppool.tile([B, RC, N], F32, tag="bl")
            for it in range(num_routing):
                if it == 0:
                    s_raw = spool.tile([B, RC, 1], F32, tag="sr")
                    nc.vector.reduce_sum(s_raw, uh, mybir.AxisListType.X)
                    scale = 1.0 / N
                else:
                    e = rpool.tile([B, RC, N], F32, tag="e")
                    nc.scalar.activation(e, bl_ps, mybir.ActivationFunctionType.Exp)
                    esum = spool.tile([B, RC, 1], F32, tag="es")
                    nc.vector.reduce_sum(esum, e, mybir.AxisListType.X)
                    erec = spool.tile([B, RC, 1], F32, tag="er")
                    nc.vector.reciprocal(erec, esum)
                    cu = rpool.tile([B, RC, N], F32, tag="cu")
                    nc.vector.tensor_mul(cu, e, uh)
                    s_raw = spool.tile([B, RC, 1], F32, tag="sr")
                    csum = spool.tile([B, RC, 1], F32, tag="cs")
                    nc.vector.reduce_sum(csum, cu, mybir.AxisListType.X)
                    nc.vector.tensor_mul(s_raw, csum, erec)
                    scale = 1.0

                # squash: v = s*|s| / (1 + s^2), s = s_raw*scale
                # critical path: Square -> +1 -> recip -> t1 -> v
                # (Abs and s_sc hang off s_raw in parallel)
                m = spool.tile([B, RC, 1], F32, tag="m")
                nc.scalar.activation(
                    m, s_raw, mybir.ActivationFunctionType.Abs, scale=scale
                )
                msq = spool.tile([B, RC, 1], F32, tag="mq")
                nc.scalar.activation(
                    msq, s_raw, mybir.ActivationFunctionType.Square, scale=scale
                )
                if scale != 1.0:
                    s_sc = spool.tile([B, RC, 1], F32, tag="ssc")
                    nc.scalar.mul(s_sc, s_raw, scale)
                else:
                    s_sc = s_raw
                den = spool.tile([B, RC, 1], F32, tag="dn")
                nc.vector.tensor_scalar_add(den, msq, 1.0)
                rec = spool.tile([B, RC, 1], F32, tag="rc")
                nc.vector.reciprocal(rec, den)
                t1 = spool.tile([B, RC, 1], F32, tag="t1")
                nc.vector.tensor_mul(t1, m, rec)
                v = spool.tile([B, RC, 1], F32, tag="v")
                nc.vector.tensor_mul(v, t1, s_sc)

                if it < num_routing - 1:
                    uv = rpool.tile([B, RC, N], mybir.dt.float16, tag="uv")
                    nc.vector.tensor_mul(uv, uh, v.to_broadcast((B, RC, N)))
                    # ones_sb holds N/B, so this accumulates
                    # bl += (N/B) * sum_b uh*v directly in PSUM
                    nc.tensor.matmul(
                        bl_ps,
                        ones_sb,
                        uv,
                        start=(it == 0),
                        stop=(it == num_routing - 2),
                    )
                else:
                    vo = spool.tile([B, RC], F32, tag="vo")
                    nc.vector.tensor_copy(out=vo, in_=v[:, :, 0])
                    nc.sync.dma_start(out=vout_slice, in_=vo)

        # Routing-chunk schedule: smaller chunks late so the final routing
        # chain (which trails the last DMA) is short.
        sched = {0: [(0, 32), (32, 32)], 1: [(0, 32), (32, 16), (48, 16)]}

        for h in range(H_LOC):
            ps = None
            uh = None
            chunk = dict()
            for c0, sz in sched[h]:
                for d in range(c0, c0 + sz):
                    chunk[d] = (c0, sz)
            for dg in range(D // DG):
                x_t = xpool.tile([128, DG, C * B], IN_DT)
                nc.sync.dma_start(
                    out=x_t,
                    in_=xt[h, dg * DG : (dg + 1) * DG].rearrange("d p f -> p d f"),
                )
                w_t = wpool.tile([128, DG, C * N], IN_DT)
                nc.sync.dma_start(
                    out=w_t,
                    in_=wt[h, dg * DG : (dg + 1) * DG].rearrange("d p f -> p d f"),
                )
                for dl in range(DG):
                    d = dg * DG + dl
                    c0, sz = chunk[d]
                    if d == c0:
                        ps = pspool.tile([B, sz, N], F32, tag="ps")
                        uh = uhpool.tile([B, sz, N], F32, tag="uh")
                    for c in range(C):
                        nc.tensor.matmul(
                            ps[:, d - c0, :],
                            x_t[:, dl, c * B : (c + 1) * B],
                            w_t[:, dl, c * N : (c + 1) * N],
                            start=(c == 0),
                            stop=(c == C - 1),
                        )
                    if d == c0 + sz - 1:
                        nc.vector.tensor_copy(out=uh, in_=ps)
                        r0 = h * D + c0
                        routing(uh, vout[:, r0 : r0 + sz], sz)
    nc.finalize()
    return nc


def _prep_core(x, W, k):
    xs = x[2 * k : 2 * k + 2]  # [2, B, S, D]
    xt = np.ascontiguousarray(
        xs.reshape(H_LOC, B, C, 128, D).transpose(0, 4, 3, 2, 1).astype(IN_NP)
    ).reshape(H_LOC, D, 128, C * B)
    ws = W[2 * k : 2 * k + 2]  # [2, D, N, S]
    wt = np.ascontiguousarray(
        ws.reshape(H_LOC, D, N, C, 128).transpose(0, 1, 4, 3, 2).astype(IN_NP)
    ).reshape(H_LOC, D, 128, C * N)
    return xt, wt


def kernel(x, W, num_routing):
    x = np.asarray(x, dtype=np.float32)
    W = np.asarray(W, dtype=np.float32)
    nr = int(num_routing)
    if nr not in _cache:
        _cache[nr] = _build(nr)
    nc = _cache[nr]

    ones = np.full((B, B), float(N) / B, dtype=np.float16)
    in_maps = []
    for k in range(NCORES):
        xt, wt = _prep_core(x, W, k)
        in_maps.append({"xt": xt, "wt": wt, "ones": ones})

    kernel.last_in_maps = in_maps
    res = run_bass_kernel_spmd(
        nc,
        in_maps,
        core_ids=list(range(NCORES)),
        trace=bool(int(os.environ.get("KERNEL_TRACE", "0"))),
    )
    kernel.last_result = res

    v_full = np.empty((H, B, D), dtype=np.float32)
    for k in range(NCORES):
        r = res.results[k]["vout"]  # [B, H_LOC*D]
        v_full[2 * k] = r[:, 0:D]
        v_full[2 * k + 1] = r[:, D : 2 * D]
    out = np.broadcast_to(
        v_full.transpose(1, 2, 0)[:, :, None, :], (B, D, N, H)
    )
    return np.ascontiguousarray(out)



# revision 35
# speedup vs baseline: 763.6182x; 763.6182x over previous
"""Trainium2 Bass kernel for nn_CapsuleSubLayer (capsule routing).

Math (per head h):
  uh[b,d,j] = sum_s W[h,d,j,s] * x[h,b,s,d]            (batched matmul over d)
  3 routing iterations of softmax / weighted-sum / squash / logit update
  out[b,d,n,h] = v[h,b,d]  (broadcast over n)

Sharding: heads are fully independent -> 2 heads per NeuronCore on 8 cores.
Host-side we pre-permute x and W into DMA-friendly per-head layouts with the
SBUF partition dim (s-chunk position p) outermost and everything else
contiguous per partition line (cast to fp16; PSUM accumulation stays fp32):
  xt[h,p,d,c*64+b] = x[h,b,c*128+p,d]   (8KB contiguous per line per d-group)
  wt[h,p,d,c*16+n] = W[h,d,n,c*128+p]   (2KB contiguous per line per d-group)
so each per-(h,d) matmul chunk is lhsT=[p,b] (stationary), rhs=[p,n] (moving),
accumulated over c=0..7 into PSUM out[b,n].

Scheduling: the PE queue carries only uh-matmuls plus the per-iteration
logit-update matmuls, and routing is emitted one *iteration slice* at a time
interleaved with the d-group loop, so every engine FIFO stays in chronological
(data-arrival) order and the DMA stream never stalls behind a routing chain.
vout stores go out on the otherwise-idle Pool queue.
"""

import os
import sys

import numpy as np

for _p in ("/opt/trn_rl_repo",):
    if _p not in sys.path:
        sys.path.insert(0, _p)

from contextlib import ExitStack

import concourse.bass as bass
import concourse.tile as tile
from concourse import bacc, mybir
from concourse.bass_utils import run_bass_kernel_spmd

F32 = mybir.dt.float32
F16 = mybir.dt.float16
BF16 = mybir.dt.bfloat16
AF = mybir.ActivationFunctionType

H, B, S, D, N = 16, 64, 1024, 64, 16
NCORES = 8
H_LOC = H // NCORES  # 2 heads per core
C = S // 128  # 8 contraction chunks

USE_FP32 = bool(int(os.environ.get("KERNEL_FP32", "0")))
IN_DT = F32 if USE_FP32 else F16
IN_NP = np.float32 if USE_FP32 else np.float16

_cache = {}


def _build(num_routing: int, repeat: int = 1, unroll: int = 1):
    nc = bacc.Bacc(
        "TRN2", target_bir_lowering=False, debug=False, num_devices=NCORES
    )
    xt = nc.dram_tensor("xt", [H_LOC, 128, D, C * B], IN_DT, kind="ExternalInput").ap()
    wt = nc.dram_tensor("wt", [H_LOC, 128, D, C * N], IN_DT, kind="ExternalInput").ap()
    ones = nc.dram_tensor("ones", [B, B], F16, kind="ExternalInput").ap()
    vout = nc.dram_tensor("vout", [B, H_LOC * D], F32, kind="ExternalOutput").ap()

    DG = 8  # d's per DMA batch

    # Routing chunks per head: two 32-d chunks per head (4 chains per body)
    # measured fastest on HW — fewer chains means less per-chain fixed cost,
    # and with ps/bl at 4 buffers each the full 8 PSUM banks stay decoupled.
    sched = {0: [(0, 32), (32, 32)], 1: [(0, 32), (32, 32)]}

    with ExitStack() as ctx:
        tc = ctx.enter_context(tile.TileContext(nc))
        xpool = ctx.enter_context(tc.tile_pool(name="xp", bufs=6))
        wpool = ctx.enter_context(tc.tile_pool(name="wp", bufs=6))
        pspool = ctx.enter_context(tc.tile_pool(name="ps", bufs=4, space="PSUM"))
        blpool = ctx.enter_context(tc.tile_pool(name="bp", bufs=1, space="PSUM"))
        rpool = ctx.enter_context(tc.tile_pool(name="rt", bufs=2))
        spool = ctx.enter_context(tc.tile_pool(name="sm", bufs=2))
        singles = ctx.enter_context(tc.tile_pool(name="sg", bufs=1))

        ones_sb = singles.tile([B, B], F16)
        nc.sync.dma_start(out=ones_sb, in_=ones)

        rep_ctx = tc.For_i(0, repeat) if repeat > 1 else None
        if rep_ctx is not None:
            ctx.enter_context(rep_ctx)

        def make_routing(ps, vout_slice, RC, key):
            """Generator emitting one routing *op* per next() call (yields True
            at iteration boundaries).  The serial chain stays entirely on DVE
            (same-engine deps issue back-to-back); Act only does Exp and the
            uh f16 copy, both off the critical path.  b_logits live in PSUM
            (bl), accumulated by ones*N/B matmuls; uh is read from PSUM for
            iter 0 and from the f16 SBUF copy afterwards (2x DVE rate).

            Division-free softmax+squash: with t = sum_n e*uh, E = sum_n e,
            s = t/E and v = s|s|/(1+s^2) simplifies to v = t|t| / (E^2+t^2);
            for the uniform iter-0 softmax, v = t|t| / (N^2+t^2)."""

            def gen():
                bl = blpool.tile([B, RC, N], F32, tag="bl", bufs=4)
                for it in range(num_routing):
                    if it == 0:
                        t = spool.tile([B, RC, 1], F32, tag=f"t{key}")
                        nc.vector.reduce_sum(t, ps, mybir.AxisListType.X)
                        yield
                        tsq = spool.tile([B, RC, 1], F32, tag=f"tq{key}")
                        nc.vector.tensor_mul(tsq, t, t)
                        yield
                        den = spool.tile([B, RC, 1], F32, tag=f"dn{key}")
                        nc.vector.tensor_scalar_add(den, tsq, float(N * N))
                        yield
                    else:
                        e = rpool.tile([B, RC, N], F32, tag=f"e{key}")
                        nc.scalar.activation(e, bl, AF.Exp)
                        yield
                        cu = rpool.tile([B, RC, N], F32, tag=f"cu{key}")
                        nc.vector.tensor_mul(cu, e, ps)
                        yield
                        t = spool.tile([B, RC, 1], F32, tag=f"t{key}")
                        nc.vector.reduce_sum(t, cu, mybir.AxisListType.X)
                        yield
                        esum = spool.tile([B, RC, 1], F32, tag=f"es{key}")
                        nc.vector.reduce_sum(esum, e, mybir.AxisListType.X)
                        yield
                        tsq = spool.tile([B, RC, 1], F32, tag=f"tq{key}")
                        nc.vector.tensor_mul(tsq, t, t)
                        yield
                        esq = spool.tile([B, RC, 1], F32, tag=f"eq{key}")
                        nc.vector.tensor_mul(esq, esum, esum)
                        yield
                        den = spool.tile([B, RC, 1], F32, tag=f"dn{key}")
                        nc.vector.tensor_add(den, esq, tsq)
                        yield
                    neg = spool.tile([B, RC, 1], F32, tag=f"ng{key}")
                    nc.vector.tensor_scalar_mul(neg, t, -1.0)
                    yield
                    m = spool.tile([B, RC, 1], F32, tag=f"m{key}")
                    nc.vector.tensor_max(m, t, neg)
                    yield
                    rec = spool.tile([B, RC, 1], F32, tag=f"rc{key}")
                    nc.vector.reciprocal(rec, den)
                    yield
                    u = spool.tile([B, RC, 1], F32, tag=f"u{key}")
                    nc.vector.tensor_mul(u, t, m)
                    yield
                    last = it == num_routing - 1
                    v = spool.tile([B, RC, 1], F32 if last else F16, tag=f"v{last}{key}")
                    nc.vector.tensor_mul(v, u, rec)
                    yield
                    if not last:
                        uv = rpool.tile([B, RC, N], F16, tag=f"uv{key}")
                        nc.vector.tensor_mul(uv, ps, v.to_broadcast((B, RC, N)))
                        yield
                        # ones_sb holds N/B, so this accumulates
                        # bl += (N/B) * sum_b uh*v directly in PSUM
                        nc.tensor.matmul(
                            bl,
                            ones_sb,
                            uv,
                            start=(it == 0),
                            stop=(it == max(num_routing - 2, 0)),
                        )
                        yield True
                    else:
                        nc.gpsimd.dma_start(out=vout_slice, in_=v[:, :, 0])
                        yield True

            return gen()

        pending = []

        def pump(iters):
            done = 0
            while done < iters and pending:
                g = pending[0]
                try:
                    if next(g) is True:
                        done += 1
                except StopIteration:
                    pending.pop(0)

        def emit_body():
            pending.clear()
            for h in range(H_LOC):
                chunk = dict()
                for c0, sz in sched[h]:
                    for d in range(c0, c0 + sz):
                        chunk[d] = (c0, sz)
                ps = None
                for dg in range(D // DG):
                    x_t = xpool.tile([128, DG, C * B], IN_DT)
                    nc.sync.dma_start(out=x_t, in_=xt[h, :, dg * DG : (dg + 1) * DG])
                    w_t = wpool.tile([128, DG, C * N], IN_DT)
                    nc.sync.dma_start(out=w_t, in_=wt[h, :, dg * DG : (dg + 1) * DG])
                    for dl in range(DG):
                        d = dg * DG + dl
                        c0, sz = chunk[d]
                        if d == c0:
                            ps = pspool.tile([B, sz, N], F32, tag="ps")
                        for c in range(C):
                            nc.tensor.matmul(
                                ps[:, d - c0, :],
                                x_t[:, dl, c * B : (c + 1) * B],
                                w_t[:, dl, c * N : (c + 1) * N],
                                start=(c == 0),
                                stop=(c == C - 1),
                            )
                        if d == c0 + sz - 1:
                            key = f"{h}_{c0}"
                            r0 = h * D + c0
                            g = make_routing(ps, vout[:, r0 : r0 + sz], sz, key)
                            next(g)  # iter-0 t reduce
                            pending.append(g)
                    pump(2)

            # flush remaining routing iterations, round-robin across chains
            while pending:
                for g in list(pending):
                    try:
                        next(g)
                    except StopIteration:
                        pending.remove(g)

        for _ in range(unroll):
            emit_body()
    nc.finalize()
    return nc


def _prep_core(x, W, k):
    xs = x[2 * k : 2 * k + 2]  # [2, B, S, D]
    xt = np.ascontiguousarray(
        xs.reshape(H_LOC, B, C, 128, D).transpose(0, 3, 4, 2, 1).astype(IN_NP)
    ).reshape(H_LOC, 128, D, C * B)
    ws = W[2 * k : 2 * k + 2]  # [2, D, N, S]
    wt = np.ascontiguousarray(
        ws.reshape(H_LOC, D, N, C, 128).transpose(0, 4, 1, 3, 2).astype(IN_NP)
    ).reshape(H_LOC, 128, D, C * N)
    return xt, wt


def kernel(x, W, num_routing):
    x = np.asarray(x, dtype=np.float32)
    W = np.asarray(W, dtype=np.float32)
    nr = int(num_routing)
    if nr not in _cache:
        _cache[nr] = _build(nr)
    nc = _cache[nr]

    ones = np.full((B, B), float(N) / B, dtype=np.float16)
    in_maps = []
    for k in range(NCORES):
        xt, wt = _prep_core(x, W, k)
        in_maps.append({"xt": xt, "wt": wt, "ones": ones})

    kernel.last_in_maps = in_maps
    res = run_bass_kernel_spmd(
        nc,
        in_maps,
        core_ids=list(range(NCORES)),
        trace=bool(int(os.environ.get("KERNEL_TRACE", "0"))),
    )
    kernel.last_result = res

    v_full = np.empty((H, B, D), dtype=np.float32)
    for k in range(NCORES):
        r = res.results[k]["vout"]  # [B, H_LOC*D]
        v_full[2 * k] = r[:, 0:D]
        v_full[2 * k + 1] = r[:, D : 2 * D]
    out = np.broadcast_to(
        v_full.transpose(1, 2, 0)[:, :, None, :], (B, D, N, H)
    )
    return np.ascontiguousarray(out)
